# revision 1
# baseline (speedup 1.0000x reference)
"""Distributed TRN2 kernel for nn_AgnosticResidualInteractionBlock.

Strategy (8 NeuronCores, SPMD via jax.pmap on the neuron PJRT backend):
  - Edges are sharded BY RECEIVER: core k owns receivers [k*1250, (k+1)*1250).
    Each core computes the complete message rows for its node slice, so no
    all-reduce is needed (the sharding_hint's all-reduce is replaced by a
    receiver-partitioned local segment-sum).
  - Within a core, edges are sorted by receiver and padded to a fixed
    per-receiver degree K_SLOT, turning the segment_sum into a dense
    reshape+sum (no scatter op on device).
  - Node-wise linears (skip connection, W_lin, W_out) are data-parallel over
    the same node slices.
  - Dummy slots carry zero edge_feats and zero edge_attrs: the bias-free silu
    MLP maps 0 -> 0, and e_s/e_v are zero, so padded slots contribute zero.
  - All device ops are kept strictly 2-D (matmul / broadcast-mul / reshape-
    sum); spherical-vector components travel as separate [*, C] arrays and the
    final (o, i) interleave is done on host. All scalar normalizations are
    folded into the weight matrices on host.

kernel(**inputs) accepts the FULL inputs and returns (message, sc) exactly
like the reference.
"""

import numpy as np

N, E, C, A, F, H = 10000, 160000, 128, 10, 8, 64
AVG_NEIGH = 16.0
NCORES = 8
NPC = N // NCORES  # 1250 nodes per core

_jax_cache = {}


def _get_jax():
    if "jax" not in _jax_cache:
        import jax
        import jax.numpy as jnp

        _jax_cache["jax"] = jax
        _jax_cache["jnp"] = jnp
    return _jax_cache["jax"], _jax_cache["jnp"]


def _core_fn(args):
    """Per-core SPMD body. All tensors are this core's shard; strictly 2-D."""
    jax, jnp = _get_jax()
    (na, nfs, nfv0, nfv1, nfv2,
     ef, es, ev0, ev1, ev2,
     sxs, sxv0, sxv1, sxv2,
     Wsc_s, Wsc_v, Wlin_s, Wlin_v,
     m0, m1, m2, m3,
     Wout_sa, Wout_sb, Wout_va, Wout_vb, Wout_vc) = args

    npc = na.shape[0]
    nslot = ef.shape[0]
    k_slot = nslot // npc

    def seg(x):  # [nslot, C] -> [npc, C]
        return x.reshape(npc, k_slot, x.shape[1]).sum(axis=1)

    # --- skip connection (scales pre-folded into Wsc_*) ---
    tp_s = (nfs[:, :, None] * na[:, None, :]).reshape(npc, C * A)
    sc_s = tp_s @ Wsc_s
    scv = []
    for nfvi in (nfv0, nfv1, nfv2):
        tp_i = (nfvi[:, :, None] * na[:, None, :]).reshape(npc, C * A)
        scv.append(tp_i @ Wsc_v)

    # --- per-slot sender features through the node linear ---
    xs = sxs @ Wlin_s
    xv0 = sxv0 @ Wlin_v
    xv1 = sxv1 @ Wlin_v
    xv2 = sxv2 @ Wlin_v

    # --- radial MLP (scales folded into m0..m3) ---
    h = jax.nn.silu(ef @ m0)
    h = jax.nn.silu(h @ m1)
    h = jax.nn.silu(h @ m2)
    tpw = h @ m3                        # [nslot, 5C]
    w1 = tpw[:, 0 * C:1 * C]
    w2 = tpw[:, 1 * C:2 * C]
    w3 = tpw[:, 2 * C:3 * C]
    w4 = tpw[:, 3 * C:4 * C]
    w5 = tpw[:, 4 * C:5 * C]

    # --- weighted CG tensor product, all 2-D ---
    ms_a = w1 * xs * es                                   # 0e x 0e
    ms_b = w4 * (xv0 * ev0 + xv1 * ev1 + xv2 * ev2)       # 1o x 1o -> 0e (1/sqrt3 in Wout_sb)
    t2 = w2 * xs
    w3es = w3 * es
    mv_a = (t2 * ev0, t2 * ev1, t2 * ev2)                 # 0e x 1o
    mv_b = (w3es * xv0, w3es * xv1, w3es * xv2)           # 1o x 0e
    mv_c = (w5 * (xv1 * ev2 - xv2 * ev1),                 # 1o x 1o -> 1o (1/sqrt2 in Wout_vc)
            w5 * (xv2 * ev0 - xv0 * ev2),
            w5 * (xv0 * ev1 - xv1 * ev0))

    # --- local segment sum + output linear (scales folded into Wout_*) ---
    out_s = seg(ms_a) @ Wout_sa + seg(ms_b) @ Wout_sb
    out_v = [seg(mv_a[i]) @ Wout_va + seg(mv_b[i]) @ Wout_vb + seg(mv_c[i]) @ Wout_vc
             for i in range(3)]

    return (out_s, out_v[0], out_v[1], out_v[2], sc_s, scv[0], scv[1], scv[2])


_compiled = {}
_capture = {}


def kernel(node_attrs, node_feats_s, node_feats_v, edge_attrs, edge_feats,
           W_sc_s, W_sc_v, W_lin_s, W_lin_v,
           mlp_w0, mlp_w1, mlp_w2, mlp_w3,
           W_out_s, W_out_v, senders, receivers):
    jax, jnp = _get_jax()

    node_attrs = np.asarray(node_attrs, np.float32)
    node_feats_s = np.asarray(node_feats_s, np.float32)
    node_feats_v = np.asarray(node_feats_v, np.float32)
    edge_attrs = np.asarray(edge_attrs, np.float32)
    edge_feats = np.asarray(edge_feats, np.float32)
    senders = np.asarray(senders)
    receivers = np.asarray(receivers)

    # ---------- host-side scale folding ----------
    inv_sc = np.float32(1.0 / np.sqrt(C * A))
    invc = np.float32(1.0 / np.sqrt(C))
    Wsc_s = np.asarray(W_sc_s, np.float32) * inv_sc
    Wsc_v = np.asarray(W_sc_v, np.float32) * inv_sc
    Wlin_s = np.asarray(W_lin_s, np.float32) * invc
    Wlin_v = np.asarray(W_lin_v, np.float32) * invc
    m0 = np.asarray(mlp_w0, np.float32) / np.sqrt(np.float32(F))
    m1 = np.asarray(mlp_w1, np.float32) / np.sqrt(np.float32(H))
    m2 = np.asarray(mlp_w2, np.float32) / np.sqrt(np.float32(H))
    m3 = np.asarray(mlp_w3, np.float32) / np.sqrt(np.float32(H))
    os_scale = np.float32(1.0 / (np.sqrt(2 * C) * AVG_NEIGH))
    ov_scale = np.float32(1.0 / (np.sqrt(3 * C) * AVG_NEIGH))
    Wo_s = np.asarray(W_out_s, np.float32) * os_scale
    Wo_v = np.asarray(W_out_v, np.float32) * ov_scale
    Wout_sa = Wo_s[:C]
    Wout_sb = Wo_s[C:] / np.sqrt(np.float32(3.0))
    Wout_va = Wo_v[0 * C:1 * C]
    Wout_vb = Wo_v[1 * C:2 * C]
    Wout_vc = Wo_v[2 * C:3 * C] / np.sqrt(np.float32(2.0))

    # ---------- host-side sharding: receiver buckets + fixed-degree slots ----
    order = np.argsort(receivers, kind="stable")
    r_sorted = receivers[order]
    s_sorted = senders[order]
    deg = np.bincount(receivers, minlength=N)
    k_slot = int(((deg.max() + 3) // 4) * 4)
    nslot = NPC * k_slot

    seg_starts = np.concatenate([[0], np.cumsum(deg)])[:-1]
    pos_in_seg = np.arange(E) - seg_starts[r_sorted]
    slot = (r_sorted % NPC) * k_slot + pos_in_seg
    core_of_edge = r_sorted // NPC

    ef_sh = np.zeros((NCORES, nslot, F), np.float32)
    ea_sh = np.zeros((NCORES, nslot, 4), np.float32)
    sxs_sh = np.zeros((NCORES, nslot, C), np.float32)
    sxv_sh = np.zeros((NCORES, 3, nslot, C), np.float32)

    ef_s = edge_feats[order]
    ea_s = edge_attrs[order]
    nfv_t = np.ascontiguousarray(node_feats_v.transpose(2, 0, 1))  # [3, N, C]
    for k in range(NCORES):
        m = core_of_edge == k
        sl = slot[m]
        ef_sh[k, sl] = ef_s[m]
        ea_sh[k, sl] = ea_s[m]
        snd = s_sorted[m]
        sxs_sh[k, sl] = node_feats_s[snd]
        for i in range(3):
            sxv_sh[k, i, sl] = nfv_t[i][snd]

    na_sh = node_attrs.reshape(NCORES, NPC, A)
    nfs_sh = node_feats_s.reshape(NCORES, NPC, C)
    nfv_sh = np.ascontiguousarray(
        node_feats_v.reshape(NCORES, NPC, C, 3).transpose(0, 3, 1, 2))  # [8,3,NPC,C]

    def rep(w):
        return np.broadcast_to(np.asarray(w, np.float32), (NCORES,) + w.shape)

    args = (na_sh, nfs_sh, nfv_sh[:, 0], nfv_sh[:, 1], nfv_sh[:, 2],
            ef_sh,
            np.ascontiguousarray(ea_sh[:, :, 0:1]),
            np.ascontiguousarray(ea_sh[:, :, 1:2]),
            np.ascontiguousarray(ea_sh[:, :, 2:3]),
            np.ascontiguousarray(ea_sh[:, :, 3:4]),
            sxs_sh, sxv_sh[:, 0], sxv_sh[:, 1], sxv_sh[:, 2],
            rep(Wsc_s), rep(Wsc_v), rep(Wlin_s), rep(Wlin_v),
            rep(m0), rep(m1), rep(m2), rep(m3),
            rep(Wout_sa), rep(Wout_sb), rep(Wout_va), rep(Wout_vb), rep(Wout_vc))

    key = ("pmap", nslot)
    try:
        if key not in _compiled:
            _compiled[key] = jax.pmap(lambda *a: _core_fn(a))
        fn = _compiled[key]
        outs = fn(*args)
        outs = [np.asarray(o) for o in outs]
        _capture["args"] = args
        _capture["fn"] = fn
    except Exception:
        # fallback: same math on CPU jax (correctness safety net)
        import jax as _jax

        with _jax.default_device(_jax.devices("cpu")[0]):
            cfn = _jax.jit(lambda *a: _core_fn(a))
            res = [cfn(*[a[k] for a in args]) for k in range(NCORES)]
            outs = [np.stack([np.asarray(r[j]) for r in res], 0) for j in range(8)]

    out_s, ov0, ov1, ov2, sc_s, scv0, scv1, scv2 = outs

    # host-side assembly: interleave vector components (o-major, i-minor)
    message = np.empty((N, 4 * C), np.float32)
    sc = np.empty((N, 4 * C), np.float32)
    message[:, :C] = out_s.reshape(N, C)
    sc[:, :C] = sc_s.reshape(N, C)
    mv = np.stack([ov0.reshape(N, C), ov1.reshape(N, C), ov2.reshape(N, C)], axis=-1)
    sv = np.stack([scv0.reshape(N, C), scv1.reshape(N, C), scv2.reshape(N, C)], axis=-1)
    message[:, C:] = mv.reshape(N, 3 * C)
    sc[:, C:] = sv.reshape(N, 3 * C)
    return message, sc


if __name__ == "__main__":
    import reference

    import jax as _j
    _cpu = _j.devices("cpu")[0]
    with _j.default_device(_cpu):
        inputs = reference.setup_inputs()
    inputs = {k: np.asarray(v) for k, v in inputs.items()}
    with _j.default_device(_cpu):
        exp_msg, exp_sc = reference.reference(**inputs)
    act_msg, act_sc = kernel(**inputs)
    for name, e, a in (("message", exp_msg, act_msg), ("sc", exp_sc, act_sc)):
        e = np.asarray(e)
        err = np.abs(a - e).max() / (np.abs(e).max() + 1e-9)
        print(f"{name}: rel_err={err:.3e}", flush=True)



# revision 17
# speedup vs baseline: 1.2183x; 1.2183x over previous
"""Distributed TRN2 Bass kernel for nn_AgnosticResidualInteractionBlock.

Strategy (8 NeuronCores, SPMD, one Bass/Tile program on all cores):
  - Edges sharded BY RECEIVER: core k owns nodes [k*1250, (k+1)*1250).
    Receiver-partitioned local segment-sum => no collective.
  - Within a core: 10 node-blocks of 128 nodes; each block's (receiver-
    sorted) edges are packed into 17 fixed edge-tiles of 128 slots.
  - The per-edge spherical-harmonic scalars (es, ev_i, -ev_i) are folded
    into SEVEN scaled copies of the one-hot segment matrix S on the HOST;
    the PE segment matmul applies them for free. The device tensor product
    is then just five wide `w (*) x` DVE ops per tile.
  - Sender features are pre-gathered on the host FEATURE-MAJOR per tile;
    the node linear runs per tile as 4 PE matmuls (lhsT = gathered tile).
  - The radial MLP runs feature-major on PE with stationary weights.
  - Skip connection: node_attrs are PE-ones-broadcast across partitions
    and folded into the stationary operand of a chunked matmul.
  - All heavy data is bf16 (fp32 accumulation in PSUM).

kernel(**inputs) takes FULL inputs, returns (message, sc) like the
reference. Host side only re-layouts (sort/pad/gather/transpose/fold).
"""

import numpy as np

N, E, C, A, F, H = 10000, 160000, 128, 10, 8, 64
AVG_NEIGH = 16.0
NCORES = 8
NPC = N // NCORES           # 1250
NBLK = (NPC + 127) // 128   # 10 node blocks / core
TPB = 17                    # edge tiles per node block (fixed, data-checked)
NT = NBLK * TPB             # 170 edge tiles / core
ET = TPB * 128              # 2176 edge slots per block

_cache = {}
_capture = {}


def _get_jax():
    if "jax" not in _cache:
        import jax
        import jax.numpy as jnp
        _cache["jax"] = jax
        _cache["jnp"] = jnp
    return _cache["jax"], _cache["jnp"]


# ---------------------------------------------------------------- host prep
def _host_prep(inp):
    import ml_dtypes
    BF16 = ml_dtypes.bfloat16

    na = np.asarray(inp["node_attrs"], np.float32)
    nfs = np.asarray(inp["node_feats_s"], np.float32)
    nfv = np.asarray(inp["node_feats_v"], np.float32)
    ea = np.asarray(inp["edge_attrs"], np.float32)
    ef = np.asarray(inp["edge_feats"], np.float32)
    snd = np.asarray(inp["senders"]).astype(np.int64)
    rcv = np.asarray(inp["receivers"]).astype(np.int32)

    inv_sc = np.float32(1.0 / np.sqrt(C * A))
    invc = np.float32(1.0 / np.sqrt(C))
    Wlin = np.stack([
        np.asarray(inp["W_lin_s"], np.float32) * invc,
        np.asarray(inp["W_lin_v"], np.float32) * invc,
    ]).astype(BF16)                                         # [2,128,128]
    Wsc = np.stack([
        (np.asarray(inp["W_sc_s"], np.float32) * inv_sc).reshape(C, A * C),
        (np.asarray(inp["W_sc_v"], np.float32) * inv_sc).reshape(C, A * C),
    ]).astype(BF16)                                         # [2,128c,(a,o)]
    m0 = (np.asarray(inp["mlp_w0"], np.float32) / np.sqrt(np.float32(F))).astype(BF16)
    m1 = (np.asarray(inp["mlp_w1"], np.float32) / np.sqrt(np.float32(H))).astype(BF16)
    m2 = (np.asarray(inp["mlp_w2"], np.float32) / np.sqrt(np.float32(H))).astype(BF16)
    m3 = (np.asarray(inp["mlp_w3"], np.float32) / np.sqrt(np.float32(H))).astype(BF16)
    os_s = np.float32(1.0 / (np.sqrt(2 * C) * AVG_NEIGH))
    ov_s = np.float32(1.0 / (np.sqrt(3 * C) * AVG_NEIGH))
    Wo_s = np.asarray(inp["W_out_s"], np.float32) * os_s
    Wo_v = np.asarray(inp["W_out_v"], np.float32) * ov_s
    Wsa = Wo_s[:C]
    Wsb = Wo_s[C:] / np.sqrt(np.float32(3.0))
    Wva, Wvb = Wo_v[0 * C:1 * C], Wo_v[1 * C:2 * C]
    Wvc = Wo_v[2 * C:3 * C] / np.sqrt(np.float32(2.0))
    # psum/msgT chunk order: [Sa Vb0 Vb1 Vb2 Va0 Vc1 Sb Va1 Vc2 Va2 Vc0]
    Wout = np.zeros((C, 11 * C), np.float32)
    for j, Wj in enumerate([Wsa, Wvb, Wvb, Wvb, Wva, Wvc, Wsb, Wva, Wvc, Wva, Wvc]):
        Wout[:, j * C:(j + 1) * C] = Wj
    Wout = Wout.astype(BF16)

    # feature-major node features (for gather + skip connection)
    nf_t = np.zeros((4, C, N), np.float32)
    nf_t[0] = nfs.T
    for i in range(3):
        nf_t[1 + i] = nfv[:, :, i].T
    nf_tb = nf_t.astype(BF16)

    nfown_t = np.zeros((NCORES, 4, C, NBLK * 128), BF16)
    na_own = np.zeros((NCORES, 1, NBLK * A * 128), np.float32)
    for k in range(NCORES):
        nfown_t[k, :, :, :NPC] = nf_tb[:, :, k * NPC:(k + 1) * NPC]
        nak = np.zeros((NBLK * 128, A), np.float32)
        nak[:NPC] = na[k * NPC:(k + 1) * NPC]
        na_own[k, 0] = nak.reshape(NBLK, 128, A).transpose(0, 2, 1).reshape(-1)
    na_own = na_own.astype(BF16)

    # ---- edge sort & fixed tiling ----
    order = np.argsort(rcv, kind="stable")
    r_s, s_s = rcv[order], snd[order]
    ef_s, ea_s = ef[order], ea[order]
    cuts = np.searchsorted(r_s, np.arange(0, N + 1))

    # seven scaled one-hots: es, ev0, ev1, ev2, -ev0, -ev1, -ev2
    S7 = np.zeros((NCORES, NBLK, 7, 128, ET), BF16)
    ef_all = np.zeros((NCORES, NBLK, F, ET), BF16)
    xg_all = np.zeros((NCORES, NBLK, C, TPB * 4 * 128), BF16)

    for k in range(NCORES):
        for b in range(NBLK):
            n0 = k * NPC + b * 128
            n1 = min(n0 + 128, (k + 1) * NPC)
            lo, hi = cuts[n0], cuts[n1]
            nb = hi - lo
            if nb > ET:
                return None
            rl = (r_s[lo:hi] - n0).astype(np.int64)
            e_in_t = np.arange(nb) % 128
            col = (np.arange(nb) // 128) * 128 + rl
            eak = ea_s[lo:hi]
            S7[k, b, 0, e_in_t, col] = eak[:, 0]
            for i in range(3):
                S7[k, b, 1 + i, e_in_t, col] = eak[:, 1 + i]
                S7[k, b, 4 + i, e_in_t, col] = -eak[:, 1 + i]
            ef_all[k, b, :, :nb] = ef_s[lo:hi].T.astype(BF16)
            # gathered sender features, feature-major, col (t, comp, e)
            g = nf_tb[:, :, s_s[lo:hi]]                     # [4, C, nb]
            gg = np.zeros((4, C, ET), BF16)
            gg[:, :, :nb] = g
            xg_all[k, b] = gg.reshape(4, C, TPB, 128).transpose(
                1, 2, 0, 3).reshape(C, TPB * 4 * 128)
    in_maps = []
    for k in range(NCORES):
        in_maps.append({
            "nfown_t": np.asarray(nfown_t[k]),
            "na_own": np.asarray(na_own[k]),
            "S7": np.asarray(S7[k]),
            "ef_all": np.asarray(ef_all[k]),
            "xg_all": np.asarray(xg_all[k]),
            "Wlin": Wlin, "Wsc": Wsc,
            "Wm0": m0, "Wm1": m1, "Wm2": m2, "Wm3": m3,
            "Wout": Wout,
        })
    return in_maps


# ---------------------------------------------------------------- bass prog
def _build_program():
    import contextlib

    import concourse.bacc as bacc
    import concourse.tile as tile
    from concourse import mybir
    from concourse.masks import make_identity

    dt = mybir.dt
    AF = mybir.ActivationFunctionType
    OP = mybir.AluOpType

    nc = bacc.Bacc("TRN2", debug=False, enable_asserts=False)

    def ein(name, shape, dtype):
        return nc.dram_tensor(name, list(shape), dtype, kind="ExternalInput").ap()

    nfown_t = ein("nfown_t", (4, C, NBLK * 128), dt.bfloat16)
    na_own = ein("na_own", (1, NBLK * A * 128), dt.bfloat16)
    S7 = ein("S7", (NBLK, 7, 128, ET), dt.bfloat16)
    ef_all = ein("ef_all", (NBLK, F, ET), dt.bfloat16)
    xg_all = ein("xg_all", (NBLK, C, TPB * 4 * 128), dt.bfloat16)
    Wlin = ein("Wlin", (2, C, C), dt.bfloat16)
    Wsc = ein("Wsc", (2, C, A * C), dt.bfloat16)
    Wm0 = ein("Wm0", (F, H), dt.bfloat16)
    Wm1 = ein("Wm1", (H, H), dt.bfloat16)
    Wm2 = ein("Wm2", (H, H), dt.bfloat16)
    Wm3 = ein("Wm3", (H, 5 * C), dt.bfloat16)
    Wout = ein("Wout", (C, 11 * C), dt.bfloat16)

    msg_out = nc.dram_tensor("msg_out", [NBLK, 4, C, 128], dt.float32,
                             kind="ExternalOutput").ap()
    sc_out = nc.dram_tensor("sc_out", [NBLK, 4, 128, C], dt.float32,
                            kind="ExternalOutput").ap()

    with tile.TileContext(nc) as tc, contextlib.ExitStack() as ctx:
        singles = ctx.enter_context(tc.tile_pool(name="singles", bufs=1))
        tp_t = ctx.enter_context(tc.tile_pool(name="tp_t", bufs=3))
        tp_blk = ctx.enter_context(tc.tile_pool(name="tp_blk", bufs=2))
        pm = ctx.enter_context(tc.tile_pool(name="pm", bufs=1, space="PSUM"))
        pt = ctx.enter_context(tc.tile_pool(name="pt", bufs=1, space="PSUM"))
        ph = ctx.enter_context(tc.tile_pool(name="ph", bufs=1, space="PSUM"))
        px = ctx.enter_context(tc.tile_pool(name="px", bufs=1, space="PSUM"))
        ps = ctx.enter_context(tc.tile_pool(name="ps", bufs=1, space="PSUM"))

        # ------- constants in SBUF -------
        w0_sb = singles.tile([F, H], dt.bfloat16)
        nc.sync.dma_start(out=w0_sb, in_=Wm0)
        w1_sb = singles.tile([H, H], dt.bfloat16)
        nc.sync.dma_start(out=w1_sb, in_=Wm1)
        w2_sb = singles.tile([H, H], dt.bfloat16)
        nc.sync.dma_start(out=w2_sb, in_=Wm2)
        w3_sb = singles.tile([H, 5 * C], dt.bfloat16)
        nc.sync.dma_start(out=w3_sb, in_=Wm3)
        wout_sb = singles.tile([C, 11 * C], dt.bfloat16)
        nc.sync.dma_start(out=wout_sb, in_=Wout)
        wlin_sb = singles.tile([C, 2 * C], dt.bfloat16)
        nc.sync.dma_start(out=wlin_sb[:, 0:C], in_=Wlin[0])
        nc.sync.dma_start(out=wlin_sb[:, C:2 * C], in_=Wlin[1])
        wsc_sb = singles.tile([C, 2 * A * C], dt.bfloat16)
        nc.sync.dma_start(out=wsc_sb[:, :A * C], in_=Wsc[0])
        nc.sync.dma_start(out=wsc_sb[:, A * C:], in_=Wsc[1])
        na_sb = singles.tile([1, NBLK * A * 128], dt.bfloat16)
        nc.sync.dma_start(out=na_sb, in_=na_own)
        nfown_sb = singles.tile([C, 4 * NBLK * 128], dt.bfloat16)
        for c4 in range(4):
            nc.sync.dma_start(
                out=nfown_sb[:, c4 * NBLK * 128:(c4 + 1) * NBLK * 128],
                in_=nfown_t[c4])
        ident = singles.tile([128, 128], dt.bfloat16)
        make_identity(nc, ident[:])
        ones_sb = singles.tile([1, 128], dt.bfloat16)
        nc.vector.memset(ones_sb[:], 1.0)

        # ------- nab precompute: node_attrs broadcast across partitions ----
        nab_all = singles.tile([128, NBLK * A * 128], dt.bfloat16)
        for b in range(NBLK):
            for half in range(2):
                pnab = pt.tile([128, 5 * 128], dt.float32, tag="ptpw")
                for a5 in range(5):
                    o0 = (b * A + half * 5 + a5) * 128
                    nc.tensor.matmul(
                        out=pnab[:, a5 * 128:(a5 + 1) * 128],
                        lhsT=ones_sb[:], rhs=na_sb[0:1, o0:o0 + 128],
                        start=True, stop=True)
                nc.vector.tensor_copy(
                    out=nab_all[:, (b * A + half * 5) * 128:
                                (b * A + half * 5 + 5) * 128],
                    in_=pnab[:])

        # seg-matmul plan: (s7_idx, rhs_chunk_in_Mr, psum_chunk, first_writer)
        # Mr chunks: 0:P1 1:P3_0 2:P3_1 3:P3_2 4:P2 5:P4_0 6:P4_1 7:P4_2
        #            8:P5_0 9:P5_1 10:P5_2
        # psum chunks: 0:Sa 1:Vb0 2:Vb1 3:Vb2 4:Va0 5:Vc1 6:Sb 7:Va1
        #              8:Vc2 9:Va2 10:Vc0
        # NOTE: start_tensor_calc resets the ENTIRE psum bank on TRN2, so
        # exactly ONE matmul per bank carries start=True (at t==0); all other
        # chunks in that bank then accumulate onto the zeroed bank.
        SEG = [
            (0, 0, 0, 4, True),    # S_es @ [P1 P3*] -> [Sa Vb*] (bank0, N=512)
            (1, 4, 4, 1, True),    # S_ev0 @ P2   -> Va0  (bank1 zeroer)
            (1, 10, 5, 1, False),  # S_ev0 @ P5_2 -> Vc1 (+)
            (1, 5, 6, 1, False),   # S_ev0 @ P4_0 -> Sb
            (2, 4, 7, 1, False),   # S_ev1 @ P2   -> Va1
            (2, 8, 8, 1, True),    # S_ev1 @ P5_0 -> Vc2 (+) (bank2 zeroer)
            (2, 6, 6, 1, False),   # S_ev1 @ P4_1 -> Sb (+)
            (3, 4, 9, 1, False),   # S_ev2 @ P2   -> Va2
            (3, 9, 10, 1, False),  # S_ev2 @ P5_1 -> Vc0 (+)
            (3, 7, 6, 1, False),   # S_ev2 @ P4_2 -> Sb (+)
            (4, 9, 8, 1, False),   # S_evn0 @ P5_1 -> Vc2 (-)
            (5, 10, 10, 1, False),  # S_evn1 @ P5_2 -> Vc0 (-)
            (6, 8, 5, 1, False),   # S_evn2 @ P5_0 -> Vc1 (-)
        ]

        # ------- main loop: node blocks -------
        for b in range(NBLK):
            ef_sb = tp_blk.tile([F, ET], dt.bfloat16, tag="ef")
            nc.sync.dma_start(out=ef_sb, in_=ef_all[b])

            pmsg = pm.tile([128, 11 * C], dt.float32, tag="pmsg")

            for t in range(TPB):
                S_sb = tp_t.tile([128, 7 * 128], dt.bfloat16, tag="S")
                nc.sync.dma_start(
                    out=S_sb[:].rearrange("p (s e) -> p s e", s=7),
                    in_=S7[b][:, :, t * 128:(t + 1) * 128].transpose([1, 0, 2]))
                xg_sb = tp_t.tile([C, 4 * 128], dt.bfloat16, tag="xg")
                nc.sync.dma_start(
                    out=xg_sb,
                    in_=xg_all[b][:, (t * 4) * 128:(t * 4 + 4) * 128])

                # node linear on pre-gathered sender features
                pxl = px.tile([128, 4 * C], dt.float32, tag="pxl")
                for c4 in range(4):
                    nc.tensor.matmul(
                        out=pxl[:, c4 * C:(c4 + 1) * C],
                        lhsT=xg_sb[:, c4 * 128:(c4 + 1) * 128],
                        rhs=wlin_sb[:, 0:C] if c4 == 0 else wlin_sb[:, C:2 * C],
                        start=True, stop=True)
                xt = tp_t.tile([128, 4 * C], dt.bfloat16, tag="xt")
                nc.scalar.copy(out=xt[:], in_=pxl[:])

                # radial MLP, feature-major
                ph0 = ph.tile([H, 128], dt.float32, tag="ph")
                nc.tensor.matmul(out=ph0[:], lhsT=w0_sb[:],
                                 rhs=ef_sb[:, t * 128:(t + 1) * 128],
                                 start=True, stop=True)
                h0 = tp_t.tile([H, 128], dt.bfloat16, tag="h0")
                nc.scalar.activation(h0[:], ph0[:], AF.Silu)
                ph1 = ph.tile([H, 128], dt.float32, tag="ph")
                nc.tensor.matmul(out=ph1[:], lhsT=w1_sb[:], rhs=h0[:],
                                 start=True, stop=True)
                h1 = tp_t.tile([H, 128], dt.bfloat16, tag="h1")
                nc.scalar.activation(h1[:], ph1[:], AF.Silu)
                ph2 = ph.tile([H, 128], dt.float32, tag="ph")
                nc.tensor.matmul(out=ph2[:], lhsT=w2_sb[:], rhs=h1[:],
                                 start=True, stop=True)
                h2 = tp_t.tile([H, 128], dt.bfloat16, tag="h2")
                nc.scalar.activation(h2[:], ph2[:], AF.Silu)
                ptpw = pt.tile([128, 5 * C], dt.float32, tag="ptpw")
                nc.tensor.matmul(out=ptpw[:, 0:512], lhsT=h2[:],
                                 rhs=w3_sb[:, 0:512], start=True, stop=True)
                nc.tensor.matmul(out=ptpw[:, 512:640], lhsT=h2[:],
                                 rhs=w3_sb[:, 512:640], start=True, stop=True)
                tpwE = tp_t.tile([128, 5 * C], dt.bfloat16, tag="tpwE")
                nc.vector.tensor_copy(out=tpwE[:], in_=ptpw[:])

                # Mr = [P1 | P3* | P2 | P4* | P5*] ; P_i = w_i (*) x
                Mr = tp_t.tile([128, 11 * C], dt.bfloat16, tag="Mr")
                xs = xt[:, 0:C]
                xv = xt[:, C:4 * C]
                nc.vector.tensor_tensor(out=Mr[:, 0:C], in0=tpwE[:, 0:C],
                                        in1=xs, op=OP.mult)
                nc.vector.tensor_tensor(out=Mr[:, 4 * C:5 * C],
                                        in0=tpwE[:, C:2 * C], in1=xs, op=OP.mult)
                nc.vector.tensor_tensor(
                    out=Mr[:, 1 * C:4 * C].rearrange("p (r c) -> p r c", r=3),
                    in0=tpwE[:, 2 * C:3 * C].unsqueeze(1).to_broadcast([128, 3, C]),
                    in1=xv.rearrange("p (r c) -> p r c", r=3),
                    op=OP.mult)
                nc.vector.tensor_tensor(
                    out=Mr[:, 5 * C:8 * C].rearrange("p (r c) -> p r c", r=3),
                    in0=tpwE[:, 3 * C:4 * C].unsqueeze(1).to_broadcast([128, 3, C]),
                    in1=xv.rearrange("p (r c) -> p r c", r=3),
                    op=OP.mult)
                nc.vector.tensor_tensor(
                    out=Mr[:, 8 * C:11 * C].rearrange("p (r c) -> p r c", r=3),
                    in0=tpwE[:, 4 * C:5 * C].unsqueeze(1).to_broadcast([128, 3, C]),
                    in1=xv.rearrange("p (r c) -> p r c", r=3),
                    op=OP.mult)

                # segment matmuls with scaled one-hots
                for (s7i, rc, pc, nch, first) in SEG:
                    nc.tensor.matmul(
                        out=pmsg[:, pc * C:(pc + nch) * C],
                        lhsT=S_sb[:, s7i * 128:(s7i + 1) * 128],
                        rhs=Mr[:, rc * C:(rc + nch) * C],
                        start=(t == 0 and first), stop=(t == TPB - 1),
                        skip_group_check=True)

            # ---- block tail: evict msg, transpose, output linear ----
            msg_sb = tp_blk.tile([128, 11 * C], dt.bfloat16, tag="msg")
            nc.vector.tensor_copy(out=msg_sb[:, 0:6 * C], in_=pmsg[:, 0:6 * C])
            nc.scalar.copy(out=msg_sb[:, 6 * C:11 * C], in_=pmsg[:, 6 * C:11 * C])
            msgT = tp_blk.tile([128, 11 * C], dt.bfloat16, tag="msgT")
            for j in range(11):
                ptr = ps.tile([128, 512], dt.bfloat16, tag="sm")
                nc.tensor.transpose(out=ptr[:, 0:C],
                                    in_=msg_sb[:, j * C:(j + 1) * C],
                                    identity=ident[:])
                if j % 2 == 0:
                    nc.vector.tensor_copy(out=msgT[:, j * C:(j + 1) * C],
                                          in_=ptr[:, 0:C])
                else:
                    nc.scalar.copy(out=msgT[:, j * C:(j + 1) * C], in_=ptr[:, 0:C])
            outmsg = tp_blk.tile([128, 4 * C], dt.float32, tag="outmsg")
            CH = {0: (0, 6), 1: (4, 1, 10), 2: (7, 2, 5), 3: (9, 3, 8)}
            for c4 in range(4):
                chunks = CH[c4]
                pout = ps.tile([128, 512], dt.float32, tag="sm")
                for ji, j in enumerate(chunks):
                    nc.tensor.matmul(
                        out=pout[:, 0:C],
                        lhsT=wout_sb[:, j * C:(j + 1) * C],
                        rhs=msgT[:, j * C:(j + 1) * C],
                        start=(ji == 0), stop=(ji == len(chunks) - 1))
                if c4 % 2 == 0:
                    nc.vector.tensor_copy(out=outmsg[:, c4 * C:(c4 + 1) * C],
                                          in_=pout[:, 0:C])
                else:
                    nc.scalar.copy(out=outmsg[:, c4 * C:(c4 + 1) * C],
                                   in_=pout[:, 0:C])
            nc.sync.dma_start(out=msg_out[b].transpose([1, 0, 2]),
                              in_=outmsg[:].rearrange("p (c n) -> p c n", c=4))

            # ---- skip connection for this block ----
            outsc = tp_blk.tile([128, 4 * C], dt.float32, tag="outsc")
            for c4 in range(4):
                X = tp_blk.tile([C, A * 128], dt.bfloat16, tag="X")
                nfb = nfown_sb[:, (c4 * NBLK + b) * 128:(c4 * NBLK + b + 1) * 128]
                nc.vector.tensor_tensor(
                    out=X[:].rearrange("p (a n) -> p a n", a=A),
                    in0=nfb.unsqueeze(1).to_broadcast([C, A, 128]),
                    in1=nab_all[:, b * A * 128:(b + 1) * A * 128].rearrange(
                        "p (a n) -> p a n", a=A),
                    op=OP.mult)
                psc = ps.tile([128, 512], dt.float32, tag="sm")
                wsc_c = wsc_sb[:, 0:A * C] if c4 == 0 else wsc_sb[:, A * C:]
                for a in range(A):
                    nc.tensor.matmul(
                        out=psc[:, 0:C],
                        lhsT=X[:, a * 128:(a + 1) * 128],
                        rhs=wsc_c[:, a * C:(a + 1) * C],
                        start=(a == 0), stop=(a == A - 1))
                if c4 % 2 == 0:
                    nc.scalar.copy(out=outsc[:, c4 * C:(c4 + 1) * C],
                                   in_=psc[:, 0:C])
                else:
                    nc.vector.tensor_copy(out=outsc[:, c4 * C:(c4 + 1) * C],
                                          in_=psc[:, 0:C])
            nc.sync.dma_start(out=sc_out[b].transpose([1, 0, 2]),
                              in_=outsc[:].rearrange("p (c o) -> p c o", c=4))

    nc.compile()
    nc.finalize()
    return nc


# ------------------------------------------------------------- pjrt runner
def _prepare_fn(nc):
    """Build a reusable jitted shard_map callable over the bass program."""
    jax, _ = _get_jax()
    from jax.sharding import Mesh, PartitionSpec
    try:
        from jax.experimental.shard_map import shard_map
    except ImportError:
        from jax.shard_map import shard_map
    from concourse import bass2jax, mybir
    bass2jax.install_neuronx_cc_hook()

    partition_name = (nc.partition_id_tensor.name
                      if nc.partition_id_tensor else None)
    in_names, out_names, out_avals, zero_shapes = [], [], [], []
    for alloc in nc.m.functions[0].allocations:
        if not isinstance(alloc, mybir.MemoryLocationSet):
            continue
        name = alloc.memorylocations[0].name
        if alloc.kind == "ExternalInput":
            if name != partition_name:
                in_names.append(name)
        elif alloc.kind == "ExternalOutput":
            shape = tuple(alloc.tensor_shape)
            dtype = mybir.dt.np(alloc.dtype)
            out_names.append(name)
            out_avals.append(jax.core.ShapedArray(shape, dtype))
            zero_shapes.append((shape, dtype))
    all_in = list(in_names) + list(out_names)
    if partition_name is not None:
        all_in.append(partition_name)

    def _body(*args):
        operands = list(args)
        if partition_name is not None:
            operands.append(bass2jax.partition_id_tensor())
        outs = bass2jax._bass_exec_p.bind(
            *operands,
            out_avals=tuple(out_avals),
            in_names=tuple(all_in),
            out_names=tuple(out_names),
            lowering_input_output_aliases=(),
            sim_require_finite=False,
            sim_require_nnan=False,
            nc=nc,
        )
        return tuple(outs)

    devices = jax.devices()[:NCORES]
    mesh = Mesh(np.asarray(devices), ("core",))
    nin = len(in_names) + len(zero_shapes)
    fn = jax.jit(shard_map(
        _body, mesh=mesh,
        in_specs=(PartitionSpec("core"),) * nin,
        out_specs=(PartitionSpec("core"),) * len(out_names),
        check_rep=False))
    return fn, in_names, out_names, zero_shapes


def _concat_args(in_maps, in_names, zero_shapes):
    concat_in = [
        np.concatenate([np.asarray(in_maps[c][nm]) for c in range(NCORES)], 0)
        for nm in in_names
    ]
    concat_zero = [np.zeros((NCORES * s[0], *s[1:]), d) for s, d in zero_shapes]
    return tuple(concat_in + concat_zero)


# ------------------------------------------------------------------ driver
def _assemble(out_map):
    msg_out = np.asarray(out_map["msg_out"]).reshape(NCORES, NBLK, 4, C, 128)
    sc_out = np.asarray(out_map["sc_out"]).reshape(NCORES, NBLK, 4, 128, C)
    message = np.zeros((N, 4 * C), np.float32)
    sc = np.zeros((N, 4 * C), np.float32)
    for k in range(NCORES):
        for b in range(NBLK):
            n0 = k * NPC + b * 128
            n1 = min(n0 + 128, (k + 1) * NPC)
            nn = n1 - n0
            message[n0:n1, 0:C] = msg_out[k, b, 0, :, :nn].T
            for i in range(3):
                message[n0:n1, C + i::3] = msg_out[k, b, 1 + i, :, :nn].T
            sc[n0:n1, 0:C] = sc_out[k, b, 0, :nn, :]
            for i in range(3):
                sc[n0:n1, C + i::3] = sc_out[k, b, 1 + i, :nn, :]
    return message, sc


def _numpy_fallback(inp):
    na = np.asarray(inp["node_attrs"], np.float32)
    nfs = np.asarray(inp["node_feats_s"], np.float32)
    nfv = np.asarray(inp["node_feats_v"], np.float32)
    ea = np.asarray(inp["edge_attrs"], np.float32)
    ef = np.asarray(inp["edge_feats"], np.float32)
    snd = np.asarray(inp["senders"]).astype(np.int64)
    rcv = np.asarray(inp["receivers"]).astype(np.int64)
    inv = np.float32(1.0 / np.sqrt(C * A))
    invc = np.float32(1.0 / np.sqrt(C))
    tp_s = (nfs[:, :, None] * na[:, None, :]).reshape(N, C * A)
    sc_s = tp_s @ np.asarray(inp["W_sc_s"], np.float32) * inv
    tp_v = (nfv[:, :, None, :] * na[:, None, :, None]).reshape(N, C * A, 3)
    sc_v = np.einsum("nki,ko->noi", tp_v,
                     np.asarray(inp["W_sc_v"], np.float32)) * inv
    x_s = nfs @ np.asarray(inp["W_lin_s"], np.float32) * invc
    x_v = np.einsum("nci,co->noi", nfv, np.asarray(inp["W_lin_v"], np.float32)) * invc

    def silu(x):
        return x / (1.0 + np.exp(-x))
    h = silu(ef @ np.asarray(inp["mlp_w0"], np.float32) / np.sqrt(np.float32(F)))
    h = silu(h @ np.asarray(inp["mlp_w1"], np.float32) / np.sqrt(np.float32(H)))
    h = silu(h @ np.asarray(inp["mlp_w2"], np.float32) / np.sqrt(np.float32(H)))
    tpw = h @ np.asarray(inp["mlp_w3"], np.float32) / np.sqrt(np.float32(H))
    w1, w2, w3, w4, w5 = np.split(tpw, 5, axis=1)
    xs, xv = x_s[snd], x_v[snd]
    es, ev = ea[:, 0:1], ea[:, 1:4]
    m0a = w1 * xs * es
    m1a = (w2 * xs)[:, :, None] * ev[:, None, :]
    m1b = w3[:, :, None] * xv * es[:, :, None]
    m0b = w4 * np.einsum("eci,ei->ec", xv, ev) / np.sqrt(np.float32(3))
    m1c = w5[:, :, None] * np.cross(xv, ev[:, None, :]) / np.sqrt(np.float32(2))
    mid_s = np.concatenate([m0a, m0b], axis=1)
    mid_v = np.concatenate([m1a, m1b, m1c], axis=1)
    msg_s = np.zeros((N, 2 * C), np.float32)
    np.add.at(msg_s, rcv, mid_s)
    msg_v = np.zeros((N, 3 * C, 3), np.float32)
    np.add.at(msg_v, rcv, mid_v)
    out_s = (msg_s @ np.asarray(inp["W_out_s"], np.float32)
             / np.sqrt(np.float32(2 * C)) / AVG_NEIGH)
    out_v = (np.einsum("nki,ko->noi", msg_v, np.asarray(inp["W_out_v"], np.float32))
             / np.sqrt(np.float32(3 * C)) / AVG_NEIGH)
    message = np.concatenate([out_s, out_v.reshape(N, C * 3)], axis=1)
    sc = np.concatenate([sc_s, sc_v.reshape(N, C * 3)], axis=1)
    return message.astype(np.float32), sc.astype(np.float32)


def kernel(node_attrs, node_feats_s, node_feats_v, edge_attrs, edge_feats,
           W_sc_s, W_sc_v, W_lin_s, W_lin_v,
           mlp_w0, mlp_w1, mlp_w2, mlp_w3,
           W_out_s, W_out_v, senders, receivers):
    inp = dict(node_attrs=node_attrs, node_feats_s=node_feats_s,
               node_feats_v=node_feats_v, edge_attrs=edge_attrs,
               edge_feats=edge_feats, W_sc_s=W_sc_s, W_sc_v=W_sc_v,
               W_lin_s=W_lin_s, W_lin_v=W_lin_v, mlp_w0=mlp_w0, mlp_w1=mlp_w1,
               mlp_w2=mlp_w2, mlp_w3=mlp_w3, W_out_s=W_out_s, W_out_v=W_out_v,
               senders=senders, receivers=receivers)
    try:
        in_maps = _host_prep(inp)
        if in_maps is None:
            raise RuntimeError("edge tile overflow; falling back")
        if "nc" not in _cache:
            _cache["nc"] = _build_program()
        if "fn" not in _cache:
            fn, in_names, out_names, zero_shapes = _prepare_fn(_cache["nc"])
            _cache.update(fn=fn, in_names=in_names, out_names=out_names,
                          zero_shapes=zero_shapes)
        args = _concat_args(in_maps, _cache["in_names"], _cache["zero_shapes"])
        out = _cache["fn"](*args)
        out = [np.asarray(o) for o in out]
        _capture["fn"] = _cache["fn"]
        _capture["args"] = args
        out_map = {nm: out[i] for i, nm in enumerate(_cache["out_names"])}
        return _assemble(out_map)
    except Exception:
        import traceback
        traceback.print_exc()
        return _numpy_fallback(inp)


if __name__ == "__main__":
    import jax as _j
    with _j.default_device(_j.devices("cpu")[0]):
        import reference
        inputs = {k: np.asarray(v) for k, v in reference.setup_inputs().items()}
        exp_msg, exp_sc = (np.asarray(x) for x in reference.reference(**inputs))
    act_msg, act_sc = kernel(**inputs)
    for name, e, a in (("message", exp_msg, act_msg), ("sc", exp_sc, act_sc)):
        err = np.abs(a - e).max() / (np.abs(e).max() + 1e-9)
        print(f"{name}: rel_err={err:.3e}", flush=True)
